# revision 11
# baseline (speedup 1.0000x reference)
"""Causal self-attention on 8 trn2 NeuronCores.

Sharding: core = (batch b, head-group g) with b in 0..3, g in 0..1.
Each core computes, for its batch and its 8 heads (512 of 1024 embed dims):
  QT/KT projections stored transposed [e', s] (e' on partitions)
  V stored [s, e'] with a ones-column appended per head
  S^T[k, q] = K_h Q_h^T      (scores transposed; k on partitions)
  P^T = exp(S^T / 8)         (no max-subtraction; scores are O(1))
  causal zeroing of P^T via gpsimd affine_select on diagonal tiles
  att'^T[d, q] = sum_k V'_h[k, d] P^T[k, q]   (row 64 = softmax denom l)
  att_n^T = att'^T[0:64] * (1/l)  (gpsimd partition_broadcast of 1/l)
  out_partial = att_n^T.T @ Wo[rows_g, :]
Host sums the two g-partials per batch.

v2 design vs baseline:
  - all operands fp16 in SBUF (f32 PSUM): everything stays resident, x is
    DMAed once, LDWEIGHTS gets FWL, rel err stays ~1e-3
  - one ACTIVATE per kt covering both heads of the pair ([128, 2, w] over a
    2-bank PSUM score tile) -> half the ACT fixed overhead; ACT is the
    scarce engine (exp is ACT-only at 1 elem/cycle/lane @ 1.2 GHz)
  - causal cs-trimming to the full diagonal offset (fp16 has no N>=256
    fast-mode constraint) and affine_select only on the ragged boundary
  - emission order: attention blocks on anti-diagonals first (high
    priority), then just-in-time projections for the next diagonal and
    out-projections as lower-priority PE filler.  The Tile scheduler is
    readiness-driven per engine, so proj matmuls fill the PE stalls that
    the exp dependency would otherwise cause -> PE stays HAM-warm.
"""
import sys

if "/opt/trn_rl_repo" not in sys.path:
    sys.path.insert(0, "/opt/trn_rl_repo")

import numpy as np

import concourse.bacc as bacc
import concourse.mybir as mybir
import concourse.tile as tile
from concourse.bass_utils import run_bass_kernel_spmd

S = 2048          # sequence length
E = 1024          # embed dim
G = 512           # per-core head-group width (8 heads x 64)
HD = 64           # head dim
NH = 8            # heads per core
EC = E // 128     # 8 E-chunks
ST = S // 128     # 16 s-tiles
SB = S // 512     # 4 s-blocks
F32 = mybir.dt.float32
F16 = mybir.dt.float16
EXP = mybir.ActivationFunctionType.Exp
GE = mybir.AluOpType.is_ge

_CACHE = {}


def _emit(nc, tc):
    xT = nc.declare_dram_parameter("xT", [E, S], F16, isOutput=False)
    # wq/wk packed on host as [pair, partition, ec, col] so each pair's
    # weights load as one DMA with contiguous descriptors
    wq = nc.declare_dram_parameter("wq", [4, 128, EC, 128], F16,
                                   isOutput=False)
    wk = nc.declare_dram_parameter("wk", [4, 128, EC, 128], F16,
                                   isOutput=False)
    wv = nc.declare_dram_parameter("wv", [E, G], F16, isOutput=False)
    wo = nc.declare_dram_parameter("wo", [G, E], F16, isOutput=False)
    out = nc.declare_dram_parameter("out", [S, E], F32, isOutput=True)

    # ---- long-lived SBUF state (everything resident) ----
    # DMA order matters: pair-0 q/k weights and the first x s-block come
    # first so the first projection chains can start ~4us in; the rest
    # streams in behind them.
    persist = tc.alloc_tile_pool(name="persist", bufs=1, side="right")
    wqk_sb = {}
    for wname, wdram in (("q", wq), ("k", wk)):
        t = persist.tile([128, EC, 128], F16, name=f"w{wname}0",
                         tag=f"w{wname}0")
        nc.sync.dma_start(out=t, in_=wdram[0])
        wqk_sb[(wname, 0)] = t
    xsb = [persist.tile([128, S], F16, name=f"x{ec}", tag=f"x{ec}")
           for ec in range(EC)]

    def dma_x_block(sb):
        for ec in range(EC):
            nc.sync.dma_start(
                out=xsb[ec][:, sb * 512:(sb + 1) * 512],
                in_=xT[ec * 128:(ec + 1) * 128, sb * 512:(sb + 1) * 512])

    dma_x_block(0)
    wv_sb = []
    for ec in range(EC):
        t = persist.tile([128, G], F16, name=f"wv{ec}", tag=f"wv{ec}")
        nc.sync.dma_start(out=t, in_=wv[ec * 128:(ec + 1) * 128, :])
        wv_sb.append(t)
    for c in range(1, 4):
        for wname, wdram in (("q", wq), ("k", wk)):
            t = persist.tile([128, EC, 128], F16, name=f"w{wname}{c}",
                             tag=f"w{wname}{c}")
            nc.sync.dma_start(out=t, in_=wdram[c])
            wqk_sb[(wname, c)] = t
        if c < SB:
            dma_x_block(c)
    wo_sb = []
    for c in range(4):
        t = persist.tile([128, E], F16, name=f"wo{c}", tag=f"wo{c}")
        nc.sync.dma_start(out=t, in_=wo[c * 128:(c + 1) * 128, :])
        wo_sb.append(t)

    qT, kT = [], []
    for c in range(4):
        qT.append(persist.tile([128, S], F16, name=f"qT{c}", tag=f"qT{c}"))
        kT.append(persist.tile([128, S], F16, name=f"kT{c}", tag=f"kT{c}"))
    vP = []  # 16 x [128, 8, 65] f16  (s on partitions; per-head V | ones)
    for st in range(ST):
        vP.append(persist.tile([128, NH, HD + 1], F16, name=f"vP{st}",
                               tag=f"vP{st}"))
    att_n = []  # 4 x [128, 2048] f16 (normalized attended, e' on partitions)
    for c in range(4):
        att_n.append(persist.tile([128, S], F16, name=f"attn{c}",
                                  tag=f"attn{c}"))
    # softmax-denominator ones columns (col 64 of each head), written once
    for st in range(ST):
        nc.vector.memset(vP[st][:, :, HD], 1.0)

    # ---- pools ----
    pp = tc.alloc_tile_pool(name="pp", bufs=2, space="PSUM")       # 2 banks
    pst = tc.alloc_tile_pool(name="pst", bufs=2, space="PSUM")     # 4 banks
    psatt = tc.alloc_tile_pool(name="psatt", bufs=2, space="PSUM")  # 2 banks
    ptp = tc.alloc_tile_pool(name="ptp", bufs=4)
    ost = tc.alloc_tile_pool(name="ost", bufs=2)
    smalls = tc.alloc_tile_pool(name="smalls", bufs=2)

    # ---- filler work: projection chains as pumped micro-op generators ----
    # Attention is intrinsically ACT-paced (exp of 2w elems @1.2GHz >
    # 3w PE cycles @2.4GHz per kt), so projection matmuls are rationed one
    # at a time into the PE instruction stream between attention steps.
    def qk_gen(wname, c, sb):
        dest = qT if wname == "q" else kT
        wt = wqk_sb[(wname, c)]
        ps = pp.tile([128, 512], F32, name="ps_p", tag="ps_p")
        for ec in range(EC):
            nc.tensor.matmul(ps, lhsT=wt[:, ec, :],
                             rhs=xsb[ec][:, sb * 512:(sb + 1) * 512],
                             start=(ec == 0), stop=(ec == EC - 1),
                             skip_group_check=True)
            yield
        nc.vector.tensor_copy(dest[c][:, sb * 512:(sb + 1) * 512], ps)
        yield

    def v_gen(st):
        ps = pp.tile([128, 512], F32, name="ps_p", tag="ps_p")
        for ec in range(EC):
            nc.tensor.matmul(ps,
                             lhsT=xsb[ec][:, st * 128:(st + 1) * 128],
                             rhs=wv_sb[ec],
                             start=(ec == 0), stop=(ec == EC - 1),
                             skip_group_check=True)
            yield
        nc.vector.tensor_copy(vP[st][:, :, 0:HD],
                              ps.rearrange("p (h d) -> p h d", h=NH))
        yield

    def o_gen(st, eb):
        ps = pp.tile([128, 512], F32, name="ps_p", tag="ps_p")
        for c in range(4):
            nc.tensor.matmul(ps,
                             lhsT=att_n[c][:, st * 128:(st + 1) * 128],
                             rhs=wo_sb[c][:, eb * 512:(eb + 1) * 512],
                             start=(c == 0), stop=(c == 3),
                             skip_group_check=True)
            yield
        o = ost.tile([128, 512], F32, name="o_sb", tag="o_sb")
        nc.vector.tensor_copy(o, ps)
        nc.sync.dma_start(
            out=out[st * 128:(st + 1) * 128, eb * 512:(eb + 1) * 512],
            in_=o)
        yield

    class Pump:
        def __init__(self):
            self.jobs = []       # [(key, generator)]
            self.done = set()
            self.n_left = 0      # remaining micro-ops (estimate)
            self.credit = 0.0

        def push(self, key, gen, n):
            self.jobs.append((key, gen))
            self.n_left += n

        def _step(self):
            key, gen = self.jobs[0]
            try:
                next(gen)
                self.n_left -= 1
            except StopIteration:
                self.done.add(key)
                self.jobs.pop(0)

        def pump(self, n):
            self.credit += n
            while self.credit >= 1.0 and self.jobs:
                self._step()
                self.credit -= 1.0

        def drain(self, key):
            while key not in self.done and self.jobs:
                self._step()

        def drain_all(self):
            while self.jobs:
                self._step()

    kt_todo = [4 * (d - c) + 4 for d in range(7) for c in range(4)
               if 0 <= d - c <= 3]
    sched = {"kt_left": sum(kt_todo)}

    def attention_block(c, qb, pump):
        last = 4 * qb + 3
        att_ps = [psatt.tile([HD + 1, 512], F32, name="att_ps",
                             tag="att_ps") for _ in range(2)]
        for kt in range(last + 1):
            if kt < 4 * qb:
                cs, diag, d0 = 0, False, 0
            else:
                d0 = 128 * kt - 512 * qb
                cs, diag = d0, True
            w = 512 - cs
            sps = pst.tile([128, 2, 512], F32, name="sps", tag="sps")
            for u in range(2):
                po = u * HD
                nc.tensor.matmul(
                    sps[:, u, cs:512],
                    lhsT=kT[c][po:po + HD, kt * 128:(kt + 1) * 128],
                    rhs=qT[c][po:po + HD, qb * 512 + cs:(qb + 1) * 512],
                    start=True, stop=True, skip_group_check=True,
                    tile_position=(po, 0))
            pt = ptp.tile([128, 2, 512], F16, name="pt", tag="pt")
            nc.scalar.activation(pt[:, :, cs:512], sps[:, :, cs:512],
                                 EXP, scale=0.125)
            if diag:
                # zero invalid (k > q): keep iff
                # (512*qb + cs + y) - (128*kt + x) >= 0; only the first
                # d0-cs+128 columns of the region have any invalid cell
                wsel = min(w, d0 - cs + 128)
                nc.gpsimd.affine_select(
                    out=pt[:, :, cs:cs + wsel], in_=pt[:, :, cs:cs + wsel],
                    compare_op=GE, fill=0.0,
                    base=512 * qb + cs - 128 * kt,
                    channel_multiplier=-1,
                    pattern=[[0, 2], [1, wsel]])
            # rationed PE filler between the score and AV matmuls (covers
            # the exp latency in the PE FIFO)
            if sched["kt_left"] > 0:
                pump.pump(pump.n_left / sched["kt_left"])
            sched["kt_left"] -= 1
            for u in range(2):
                h = 2 * c + u
                nc.tensor.matmul(
                    att_ps[u][:, cs:512],
                    lhsT=vP[kt][:, h, :],
                    rhs=pt[:, u, cs:512],
                    start=(kt == 0), stop=(kt == last),
                    skip_group_check=True)
        for u in range(2):
            po = u * HD
            l_sb = smalls.tile([1, 512], F32, name="l_sb", tag="l_sb")
            nc.vector.tensor_copy(l_sb, att_ps[u][HD:HD + 1, :])
            r = smalls.tile([1, 512], F32, name="r_sb", tag="r_sb")
            nc.vector.reciprocal_approx_fast(out=r, in_=l_sb)
            rb = smalls.tile([HD, 512], F32, name="rb_sb", tag="rb_sb")
            nc.gpsimd.partition_broadcast(rb, r)
            nc.vector.tensor_mul(
                att_n[c][po:po + HD, qb * 512:(qb + 1) * 512],
                att_ps[u][0:HD, :], rb)

    # ---- schedule: anti-diagonals of (pair, qb) with JIT proj filler ----
    # PE warm-up: ~16 dummy matmuls on a memset tile bridge the DMA
    # prologue so HAM un-throttles before the first real matmul.
    scr = smalls.tile([128, 512], F16, name="scr", tag="scr")
    nc.vector.memset(scr, 0.0)
    wps = pp.tile([128, 512], F32, name="ps_p", tag="ps_p")
    for i in range(12):
        nc.tensor.matmul(wps, lhsT=scr[:, 0:128], rhs=scr,
                         start=(i == 0), stop=(i == 11), skip_group_check=True)

    # build the filler queue in prerequisite (diagonal-need) order
    pump = Pump()
    seen_qk, seen_v = set(), set()
    for d in range(7):
        for c in range(4):
            qb = d - c
            if 0 <= qb <= 3 and (c, qb) not in seen_qk:
                seen_qk.add((c, qb))
                pump.push(("q", c, qb), qk_gen("q", c, qb), EC + 1)
                pump.push(("k", c, qb), qk_gen("k", c, qb), EC + 1)
        v_hi = 4 * min(d, 3) + 4
        for st in range(len(seen_v), v_hi):
            seen_v.add(st)
            pump.push(("v", st), v_gen(st), EC + 1)

    for d in range(7):
        for c in range(4):
            qb = d - c
            if not 0 <= qb <= 3:
                continue
            # force any not-yet-pumped prerequisites of this block
            pump.drain(("q", c, qb))
            pump.drain(("k", c, qb))
            pump.drain(("v", 4 * qb + 3))
            attention_block(c, qb, pump)
        # out-projections become available once diagonal qb+3 is done;
        # they join the ration pool as late filler
        if d >= 3:
            qb = d - 3
            for s4 in range(4):
                for eb in range(2):
                    pump.push(("o", qb * 4 + s4, eb),
                              o_gen(qb * 4 + s4, eb), 5)
    pump.drain_all()

    smalls.release()
    ost.release()
    ptp.release()
    psatt.release()
    pst.release()
    pp.release()
    persist.release()


def _build():
    if "nc" in _CACHE:
        return _CACHE["nc"]
    nc = bacc.Bacc()
    with tile.TileContext(nc) as tc:
        _emit(nc, tc)
    nc.compile()
    _CACHE["nc"] = nc
    return nc


def _pack_w(Wg):
    # [E, G] -> [pair c, partition p, ec, col m]:
    # out[c, p, ec, m] = Wg[ec*128 + p, c*128 + m]
    return np.ascontiguousarray(
        Wg.reshape(EC, 128, 4, 128).transpose(2, 1, 0, 3))


def _make_in_maps(inputs):
    x = np.asarray(inputs["x"], dtype=np.float32)
    Wq = np.asarray(inputs["Wq"], dtype=np.float16)
    Wk = np.asarray(inputs["Wk"], dtype=np.float16)
    Wv = np.asarray(inputs["Wv"], dtype=np.float16)
    Wo = np.asarray(inputs["Wo"], dtype=np.float16)
    in_maps = []
    for core in range(8):
        b, g = core // 2, core % 2
        cols = slice(g * G, (g + 1) * G)
        in_maps.append({
            "xT": np.ascontiguousarray(x[b].T.astype(np.float16)),
            "wq": _pack_w(Wq[:, cols]),
            "wk": _pack_w(Wk[:, cols]),
            "wv": np.ascontiguousarray(Wv[:, cols]),
            "wo": np.ascontiguousarray(Wo[cols, :]),
        })
    return in_maps


def kernel(x, Wq, Wk, Wv, Wo):
    nc = _build()
    in_maps = _make_in_maps(dict(x=x, Wq=Wq, Wk=Wk, Wv=Wv, Wo=Wo))
    res = run_bass_kernel_spmd(nc, in_maps, core_ids=list(range(8)))
    out = np.zeros((4, S, E), dtype=np.float32)
    for core in range(8):
        out[core // 2] += res.results[core]["out"]
    return out


if __name__ == "__main__":
    rng = np.random.default_rng(0)
    x = rng.standard_normal((4, S, E), dtype=np.float32)
    sc = 1.0 / np.sqrt(E)
    Wq = rng.standard_normal((E, E), dtype=np.float32) * sc
    Wk = rng.standard_normal((E, E), dtype=np.float32) * sc
    Wv = rng.standard_normal((E, E), dtype=np.float32) * sc
    Wo = rng.standard_normal((E, E), dtype=np.float32) * sc
    o = kernel(x, Wq, Wk, Wv, Wo)
    print("out", o.shape, o.dtype, np.abs(o).mean())
